# revision 54
# baseline (speedup 1.0000x reference)
"""Trainium2 Bass kernel for AutoregressiveConvLSTM log-prob.

Strategy (v2)
-------------
Data-parallel over batch: 64 images -> 8 NeuronCores, 8 images each.

Layout: each plane is [H=128 partitions, FREE] where image b occupies
flat columns OFF+130*b .. OFF+130*b+129 (interior at +1..+128, one zero
pad column each side; OFF=2 leading zeros allow dx=-2 taps).

All 3x3 convs run on the TensorEngine as banded matmuls in fp8(e4m3)
with MatmulPerfMode.DoubleRow: each instruction computes
  psum += bandA.T @ movingA + bandB.T @ movingB
at 0.5 PE cycles per output column (4x the fp32r rate).  Band pairs are
host-built [128, 2, 128] fp8 tri/penta-diagonal matrices.  The dy taps
live in the band diagonals; dx taps are free-dim column offsets into
the zero pads.  Pair sources must share one SBUF tile, so the state
pack P = [128, 6, FREE] holds (h0, h1, cf0, cf1, r0, r1) and the x
stream holds (x, x-shifted-left-1) so taps pair across dx.  The
conv_in (1->1) conv is folded into Wih as a single 5x5 conv (exact for
bci=0; interior-exact otherwise), removing the u plane entirely.

Sigmoids are computed as 0.5*tanh(x/2)+0.5 (Act tanh + DVE
tensor_scalar) so every activation comes from one table set - no
LoadActFuncSet thrash.  Gate psums are [128, 2, 512] (co-pairs fused)
so one Act op covers both features.  LSTM pointwise math runs in bf16
on DVE (2x mode); h-writes (bf16*bf16->fp8) run on the idle Pool
engine.  Per-pixel log-prob terms reduce via tensor_tensor_reduce with
the lp column as both init and accumulator.
"""

import numpy as np
import ml_dtypes

B_FULL, C, H, W, F = 64, 16, 128, 128, 2
NCORES = 8
BL = B_FULL // NCORES            # images per core
WB = W + 2                       # per-image block width incl pads
OFF = 2                          # leading zero cols (dx=-2 reach)
FREE = OFF + BL * WB + 2
HALF_LOG_2PI = 0.9189385332046727
LN_SQRT2 = 0.34657359027997264

F8 = ml_dtypes.float8_e4m3
BF16 = ml_dtypes.bfloat16

# chunks: (b0, n_imgs); psum free cols = n*130
CHUNKS = [(0, 3), (3, 3), (6, 2)]


def _nz(v):
    return float(v) != 0.0


def _pair_layout(bci, bc1, bc2, bo1, bo2, bih):
    """Ordered (key -> (offset, count)) for the band-pair DRAM tensor.
    Depends only on which biases are nonzero, so the program builder can
    mirror it without the weights."""
    gb = [_nz(bih[g]) or _nz(bci[0]) for g in range(8)]
    L = []
    for co in range(2):
        L.append((f"c1_{co}", 24 + (1 if _nz(bc1[co]) else 0)))
    for co in range(2):
        L.append((f"c2_{co}", 3 + (1 if _nz(bc2[co]) else 0)))
    for g in range(8):
        L.append((f"g{g}", 6 + (1 if gb[g] else 0)))
    for co in range(2):
        L.append((f"h1_{co}", 6 + (1 if _nz(bo1[co]) else 0)))
    for co in range(2):
        L.append((f"h2_{co}", 3))
    off = {}
    o = 0
    for k, n in L:
        off[k] = (o, n)
        o += n
    return off, o


def _band3(w3):
    b = np.zeros((H, H), np.float32)
    for dy in (-1, 0, 1):
        ar = np.arange(max(0, -dy), H - max(0, dy))
        b[ar + dy, ar] = w3[dy + 1]
    return b


def _band5(w5):
    b = np.zeros((H, H), np.float32)
    for dy in (-2, -1, 0, 1, 2):
        ar = np.arange(max(0, -dy), H - max(0, dy))
        b[ar + dy, ar] = w5[dy + 2]
    return b


def _bias_band(v):
    b = np.zeros((H, H), np.float32)
    b[0, :] = v
    return b


_ZB = np.zeros((H, H), np.float32)


def _build_bands(Wci, Wc1, Wc2, Wo1, Wo2, Wih, Whh,
                 bci, bc1, bc2, bo1, bo2, bih):
    off, total = _pair_layout(bci, bc1, bc2, bo1, bo2, bih)
    bands = np.zeros((total, H, 2, H), np.float32)
    pos = {k: o for k, (o, n) in off.items()}

    def emit(key, a, b):
        i = pos[key]
        bands[i, :, 0, :] = a
        bands[i, :, 1, :] = b
        pos[key] = i + 1

    # cond1: 16 -> 2; channel pairs (2k, 2k+1)
    for co in range(2):
        k0 = f"c1_{co}"
        for k in range(8):
            for dx in range(3):
                emit(k0, _band3(Wc1[:, dx, 2 * k, co]),
                     _band3(Wc1[:, dx, 2 * k + 1, co]))
        if _nz(bc1[co]):
            emit(k0, _bias_band(bc1[co]), _ZB)
    # cond2: 2 -> 2
    for co in range(2):
        k0 = f"c2_{co}"
        for dx in range(3):
            emit(k0, _band3(Wc2[:, dx, 0, co]), _band3(Wc2[:, dx, 1, co]))
        if _nz(bc2[co]):
            emit(k0, _bias_band(bc2[co]), _ZB)
    # gates: 5x5 composite of Wci then Wih, plus Whh
    W5 = np.zeros((5, 5, 8), np.float32)
    for co in range(8):
        for a in range(3):
            for d in range(3):
                for b in range(3):
                    for e in range(3):
                        W5[a + b, d + e, co] += (
                            Wci[a, d, 0, 0] * Wih[b, e, 0, co])
    gbias = [float(bih[co]) + float(bci[0]) * float(Wih[:, :, 0, co].sum())
             for co in range(8)]
    for co in range(8):
        k0 = f"g{co}"
        for dx in range(3):
            emit(k0, _band3(Whh[:, dx, 0, co]), _band3(Whh[:, dx, 1, co]))
        emit(k0, _band5(W5[:, 0, co]), _band5(W5[:, 1, co]))   # xbase -2
        emit(k0, _band5(W5[:, 2, co]), _band5(W5[:, 3, co]))   # xbase 0
        emit(k0, _band5(W5[:, 4, co]), _ZB)                     # xbase +2
        if _nz(bih[co]) or _nz(bci[0]):
            emit(k0, _bias_band(gbias[co]), _ZB)
    # head1: feat part + cond part of Wo1
    for co in range(2):
        k0 = f"h1_{co}"
        for dx in range(3):
            emit(k0, _band3(Wo1[:, dx, 0, co]), _band3(Wo1[:, dx, 1, co]))
        for dx in range(3):
            emit(k0, _band3(Wo1[:, dx, 2, co]), _band3(Wo1[:, dx, 3, co]))
        if _nz(bo1[co]):
            emit(k0, _bias_band(bo1[co]), _ZB)
    # head2
    for co in range(2):
        k0 = f"h2_{co}"
        for dx in range(3):
            emit(k0, _band3(Wo2[:, dx, 0, co]), _band3(Wo2[:, dx, 1, co]))
    for k, (o, n) in off.items():
        assert pos[k] == o + n, (k, pos[k], o, n)
    return np.ascontiguousarray(
        bands.astype(F8).transpose(1, 0, 2, 3)), off, total


def _build_program(bci, bc1, bc2, bo1, bo2, bih):
    import concourse.bacc as bacc
    import concourse.mybir as mybir
    import concourse.tile as tile

    f32 = mybir.dt.float32
    f8 = mybir.dt.float8e4
    bf = mybir.dt.bfloat16
    AF = mybir.ActivationFunctionType
    OP = mybir.AluOpType
    AX = mybir.AxisListType
    DR = mybir.MatmulPerfMode.DoubleRow

    off, NP = _pair_layout(bci, bc1, bc2, bo1, bo2, bih)
    n_ot = off["g0"][0]                      # one-time pairs (cond)
    n_res = NP - n_ot                        # resident pairs

    nc = bacc.Bacc("TRN2", target_bir_lowering=False, debug=False)
    xd8 = nc.dram_tensor("x8", [C - 1, 2, H, FREE], f8, kind="ExternalInput")
    xbd = nc.dram_tensor("xb", [C, H, FREE], bf, kind="ExternalInput")
    cdd = nc.dram_tensor("c8", [8, 2, H, FREE], f8, kind="ExternalInput")
    bdd = nc.dram_tensor("bands", [H, NP, 2, H], f8, kind="ExternalInput")
    od = nc.dram_tensor("out", [BL, 1], f32, kind="ExternalOutput")

    def BS(b):
        return OFF + b * WB

    with tile.TileContext(nc) as tc:
        import contextlib
        ctx = contextlib.ExitStack()
        with ctx:
            state = ctx.enter_context(tc.tile_pool(name="state", bufs=1))
            sbands = ctx.enter_context(tc.tile_pool(name="sbands", bufs=1))
            xstream = ctx.enter_context(tc.tile_pool(name="xs", bufs=4))
            bstream = ctx.enter_context(tc.tile_pool(name="bs", bufs=4))
            tmp = ctx.enter_context(tc.tile_pool(name="tmp", bufs=20))
            psum = ctx.enter_context(
                tc.tile_pool(name="psum", bufs=4, space="PSUM"))
            psumH = psum

            # resident band pairs (loaded after the cond-phase DMAs below)
            sb = sbands.tile([H, n_res, 2, H], f8, tag="sb", name="sb")

            def bp(key, j):
                o, n = off[key]
                assert j < n
                return sb[:, o - n_ot + j]

            # persistent state; slots: 0,1 h(even-step) / 2,3 cf /
            # 4,5 r / 6,7 h(odd-step)  (h ping-pongs so delayed heads can
            # read the previous step's h)
            P = state.tile([H, 8, FREE], f8, tag="P", name="P")
            nc.gpsimd.memset(P[:], 0.0)
            cst_t = state.tile([H, 2, FREE], bf, tag="c", name="c")
            nc.vector.memset(cst_t[:], 0.0)
            ones8 = state.tile([H, 2, 258], f8, tag="o8", name="o8")
            nc.vector.memset(ones8[:], 1.0)
            lp = state.tile([H, BL], f32, tag="lp", name="lp")
            nc.vector.memset(lp[:], 0.0)
            ones_f = state.tile([H, 1], f32, tag="of", name="of")
            nc.vector.memset(ones_f[:], 1.0)
            # bias cols: 0 = exp bias, 1 = final output bias
            cstv = -16.0 * 128.0 * 128.0 * (float(bo2[1]) + HALF_LOG_2PI)
            bias_t = state.tile([H, 2], f32, tag="bias", name="bias")
            nc.vector.memset(bias_t[:, 0:1], -float(bo2[1]) - LN_SQRT2)
            nc.vector.memset(bias_t[:, 1:2], cstv)

            def interior(ap_flat):
                # [p, s, NN] -> [p, s, n, 128]
                return ap_flat.rearrange("p s (b w) -> p s b w", w=WB)[
                    :, :, :, 1:129]

            def regions(NN):
                # split a chunk's columns into <=256-wide matmul regions
                # (regions need not align to images; pads absorb conv
                # bleed).  Avoid degenerate tiny regions: split evenly when
                # the remainder would be very small.
                if NN <= 256:
                    return [(0, NN)]
                if NN - 256 >= 64:
                    return [(0, 256), (256, NN - 256)]
                h = NN // 2
                return [(0, h), (h, NN - h)]

            def head(key_pfx, src_slots, Tg, b0, n, include_h=True,
                     bias_flags=(False, False)):
                # head1/head2-style group: co at dim1 of Tg.  For head1 the
                # h-independent cond-pairs (idx 3..5) are emitted FIRST so
                # they can fill PE bubbles while the h chain completes.
                NN = n * 130
                for co in range(2):
                    key = f"{key_pfx}_{co}"
                    for r0, rl in regions(NN):
                        base = BS(b0) + r0
                        out = Tg[:, co, r0:r0 + rl]
                        order = []
                        if key_pfx == "h1":
                            for dxi, dx in enumerate((-1, 0, 1)):
                                order.append((3 + dxi, 2, dx))
                            if include_h:
                                for dxi, dx in enumerate((-1, 0, 1)):
                                    order.append((dxi, src_slots[0], dx))
                        else:
                            for dxi, dx in enumerate((-1, 0, 1)):
                                order.append((dxi, src_slots[0], dx))
                        if bias_flags[co]:
                            order.append((off[key][1] - 1, None, None))
                        for k, (idx, slot, dx) in enumerate(order):
                            first = (k == 0)
                            last = (k == len(order) - 1)
                            if slot is None:
                                nc.tensor.matmul(
                                    out, bp(key, idx), ones8[:, :, 0:rl],
                                    start=first, stop=last, perf_mode=DR)
                            else:
                                nc.tensor.matmul(
                                    out, bp(key, idx),
                                    P[:, slot:slot + 2,
                                      base + dx:base + dx + rl],
                                    start=first, stop=last, perf_mode=DR)

            def head2_and_lp(b0, n, xbt, fast_tail=False):
                NN = n * 130
                eng = nc.vector if fast_tail else nc.gpsimd
                Th2 = psumH.tile([H, 2, 512], f32, tag="ps", name="ps")
                head("h2", (4, 4), Th2, b0, n)
                e = tmp.tile([H, NN], bf, tag="tw", name="e")
                nc.scalar.activation(e[:], Th2[:, 1, :NN], AF.Exp,
                                     bias=bias_t[:, 0:1], scale=-1.0)
                # copy pq1 to SBUF so the head2 psum tile releases early
                ls = tmp.tile([H, NN], bf, tag="tw", name="ls")
                nc.scalar.activation(ls[:], Th2[:, 1, :NN], AF.Identity)
                d2 = tmp.tile([H, NN], bf, tag="tw", name="d2")
                nc.vector.tensor_tensor(
                    d2[:], xbt[:, BS(b0):BS(b0) + NN], Th2[:, 0, :NN],
                    OP.subtract)
                z = tmp.tile([H, NN], bf, tag="tw", name="z")
                eng.tensor_tensor(z[:], d2[:], e[:], OP.mult)
                z2 = tmp.tile([H, NN], bf, tag="tw", name="z2")
                eng.tensor_tensor(z2[:], z[:], z[:], OP.mult)
                t = tmp.tile([H, NN], bf, tag="tw", name="t")
                nc.vector.tensor_tensor(t[:], z2[:], ls[:], OP.add)
                red = tmp.tile([H, n], f32, tag="tw", name="red")
                t3 = t[:].rearrange("p (b w) -> p b w", w=WB)[:, :, 1:129]
                nc.vector.reduce_sum(red[:], t3, AX.X)
                nc.vector.tensor_add(lp[:, b0:b0 + n], lp[:, b0:b0 + n],
                                     red[:])

            # ---------------- cond phase ----------------
            with tc.tile_pool(name="otb", bufs=1) as otp, \
                 tc.tile_pool(name="cstr", bufs=1) as cstr:
                ot = otp.tile([H, n_ot, 2, H], f8, tag="ot", name="ot")

                def bot(key, j):
                    o, n = off[key]
                    assert j < n
                    return ot[:, o + j]

                tc8 = state.tile([H, 2, FREE], f8, tag="tc8", name="tc8")
                nc.vector.memset(tc8[:], 0.0)

                # all 8 cond channel-pair planes resident
                cpl = cstr.tile([H, 8, 2, FREE], f8, tag="cpl", name="cpl")
                n_c1 = off["c2_0"][0]          # cond1 pairs come first
                nc.sync.dma_start(ot[:, :n_c1], bdd[:, :n_c1])
                for k in range(8):
                    nc.sync.dma_start(
                        cpl[:, k], cdd[k].rearrange("t h w -> h t w"))
                nc.sync.dma_start(ot[:, n_c1:], bdd[:, n_c1:n_ot])
                nc.sync.dma_start(sb[:], bdd[:, n_ot:])
                for ci, (b0, n) in enumerate(CHUNKS):
                    NN = n * 130
                    pc = psum.tile([H, 2, 512], f32, tag="ps", name="ps")
                    for co in range(2):
                        key = f"c1_{co}"
                        tot = off[key][1]
                        for r0, rl in regions(NN):
                            base = BS(b0) + r0
                            out = pc[:, co, r0:r0 + rl]
                            for k in range(8):
                                for dxi, dx in enumerate((-1, 0, 1)):
                                    idx = k * 3 + dxi
                                    nc.tensor.matmul(
                                        out, bot(key, idx),
                                        cpl[:, k, :,
                                            base + dx:base + dx + rl],
                                        start=(idx == 0),
                                        stop=(idx == tot - 1),
                                        perf_mode=DR)
                            if _nz(bc1[co]):
                                nc.tensor.matmul(
                                    out, bot(key, tot - 1),
                                    ones8[:, :, 0:rl],
                                    start=False, stop=True, perf_mode=DR)
                    nc.scalar.activation(
                        interior(tc8[:, :, BS(b0):BS(b0) + NN]),
                        interior(pc[:, :, :NN]), AF.Tanh)
                # cond2 -> cf slots of P
                for ci, (b0, n) in enumerate(CHUNKS):
                    NN = n * 130
                    pq = psum.tile([H, 2, 512], f32, tag="ps", name="ps")
                    for co in range(2):
                        key = f"c2_{co}"
                        tot = off[key][1]
                        for r0, rl in regions(NN):
                            base = BS(b0) + r0
                            out = pq[:, co, r0:r0 + rl]
                            for dxi, dx in enumerate((-1, 0, 1)):
                                nc.tensor.matmul(
                                    out, bot(key, dxi),
                                    tc8[:, :, base + dx:base + dx + rl],
                                    start=(dxi == 0), stop=(dxi == tot - 1),
                                    perf_mode=DR)
                            if _nz(bc2[co]):
                                nc.tensor.matmul(
                                    out, bot(key, tot - 1),
                                    ones8[:, :, 0:rl],
                                    start=False, stop=True, perf_mode=DR)
                    nc.scalar.activation(
                        interior(P[:, 2:4, BS(b0):BS(b0) + NN]),
                        interior(pq[:, :, :NN]), AF.Identity)

            # ---------------- step 0 ----------------
            xbt0 = bstream.tile([H, FREE], bf, tag="xbt", name="xbt")
            nc.sync.dma_start(xbt0[:], xbd[0])
            h1b = (_nz(bo1[0]), _nz(bo1[1]))
            for (b0, n) in CHUNKS:
                NN = n * 130
                Th1 = psumH.tile([H, 2, 512], f32, tag="ps", name="ps")
                head("h1", (0, 0), Th1, b0, n, include_h=False,
                     bias_flags=h1b)
                nc.vector.tensor_scalar(
                    interior(P[:, 4:6, BS(b0):BS(b0) + NN]),
                    interior(Th1[:, :, :NN]), 0.0, None, OP.max)
                head2_and_lp(b0, n, xbt0)

            # ---------------- steps (phase-major across chunks) ----------
            gb = [_nz(bih[g]) or _nz(bci[0]) for g in range(8)]
            pend_CD = None
            for st in range(1, 16):
                xpl = xstream.tile([H, 2, FREE], f8, tag="xpl", name="xpl")
                nc.sync.dma_start(xpl[:], xd8[st - 1].rearrange(
                    "t h w -> h t w"))
                xbt = bstream.tile([H, FREE], bf, tag="xbt", name="xbt")
                nc.sync.dma_start(xbt[:], xbd[st])
                # Software-pipelined emission: one-chunk lag between the
                # gate phase (A), pointwise chain (B), head1 (C), head2 (D)
                # so every engine's in-order queue stays fed.
                st_tnh = {}
                st_sig = {}

                hs_r = 0 if (st - 1) % 2 == 0 else 6
                hs_w = 0 if st % 2 == 0 else 6

                def phA(ci):
                    b0, n = CHUNKS[ci]
                    NN = n * 130
                    tnh = []
                    sig = {}
                    for g in range(4):
                        Tg = psum.tile([H, 2, 512], f32, tag="ps", name="ps")
                        for f01 in range(2):
                            co = 2 * g + f01
                            key = f"g{co}"
                            # x-pairs (h-independent) first: they can fill PE
                            # bubbles while the h recurrence chain completes
                            for r0, rl in regions(NN):
                                base = BS(b0) + r0
                                out = Tg[:, f01, r0:r0 + rl]
                                for xi, xb_ in enumerate((-2, 0, 2)):
                                    nc.tensor.matmul(
                                        out, bp(key, 3 + xi),
                                        xpl[:, :, base + xb_:base + xb_ + rl],
                                        start=(xi == 0), stop=False,
                                        perf_mode=DR)
                            for r0, rl in regions(NN):
                                base = BS(b0) + r0
                                out = Tg[:, f01, r0:r0 + rl]
                                for dxi, dx in enumerate((-1, 0, 1)):
                                    nc.tensor.matmul(
                                        out, bp(key, dxi),
                                        P[:, hs_r:hs_r + 2,
                                          base + dx:base + dx + rl],
                                        start=False,
                                        stop=(dxi == 2 and not gb[co]),
                                        perf_mode=DR)
                                if gb[co]:
                                    nc.tensor.matmul(
                                        out, bp(key, off[key][1] - 1),
                                        ones8[:, :, 0:rl],
                                        start=False, stop=True, perf_mode=DR)
                        t = tmp.tile([H, 2, NN], bf, tag="tw", name="tnh")
                        nc.scalar.activation(
                            t[:], Tg[:, :, :NN], AF.Tanh,
                            scale=(1.0 if g == 2 else 0.5))
                        tnh.append(t)
                        if g != 2:
                            s = tmp.tile([H, 2, NN], bf, tag="tw", name="sig")
                            nc.vector.tensor_scalar(
                                s[:], t[:], 0.5, 0.5, OP.mult, OP.add)
                            sig[g] = s
                    st_tnh[ci] = tnh
                    st_sig[ci] = sig

                def phB(ci):
                    b0, n = CHUNKS[ci]
                    NN = n * 130
                    tnh, sig = st_tnh[ci], st_sig[ci]
                    tig = tmp.tile([H, 2, NN], bf, tag="tw", name="tig")
                    nc.vector.tensor_tensor(
                        tig[:], sig[0][:], tnh[2][:], OP.mult)
                    cs = cst_t[:, :, BS(b0):BS(b0) + NN]
                    nc.vector.tensor_tensor(cs, sig[1][:], cs, OP.mult)
                    nc.vector.tensor_tensor(cs, cs, tig[:], OP.add)
                    tcc = tmp.tile([H, 2, NN], bf, tag="tw", name="tcc")
                    nc.scalar.activation(tcc[:], cs, AF.Tanh)
                    nc.vector.tensor_tensor(
                        interior(P[:, hs_w:hs_w + 2, BS(b0):BS(b0) + NN]),
                        interior(sig[3][:]), interior(tcc[:]), OP.mult)

                def phC(ci, hs_=hs_w):
                    b0, n = CHUNKS[ci]
                    NN = n * 130
                    Th1 = psumH.tile([H, 2, 512], f32, tag="ps", name="ps")
                    head("h1", (hs_, 0), Th1, b0, n, bias_flags=h1b)
                    nc.scalar.activation(
                        interior(P[:, 4:6, BS(b0):BS(b0) + NN]),
                        interior(Th1[:, :, :NN]), AF.Relu)

                def phD(ci, xbt_=None, fast_tail=False):
                    b0, n = CHUNKS[ci]
                    head2_and_lp(b0, n, xbt_ if xbt_ is not None else xbt,
                                 fast_tail=fast_tail)

                for ci in range(3):
                    phA(ci)
                for ci in range(3):
                    phB(ci)
                # heads of the PREVIOUS step (one-step lag): their PE work
                # fills this step's recurrence-chain bubble, and next-step
                # gate psum tiles can allocate before head tiles
                if pend_CD is not None:
                    pC, pD = pend_CD
                    for ci in range(3):
                        pC(ci)
                    for ci in range(3):
                        pD(ci)
                pend_CD = (phC,
                           (lambda ci, xb_=xbt, ft=False: phD(ci, xb_, ft)))
            # drain the last step's heads
            pC, pD = pend_CD
            for ci in range(3):
                pC(ci)
                pD(ci, None, True)

            # final (reuse a psum-pool tile for the [8,1] reduction)
            po_t = psumH.tile([H, 2, 512], f32, tag="ps", name="ps")
            po = po_t[:BL, 0, 0:1]
            nc.tensor.matmul(po, lp[:], ones_f[:], start=True, stop=True)
            osb = state.tile([BL, 1], f32, tag="osb", name="osb")
            nc.scalar.activation(osb[:], po, AF.Identity,
                                 scale=-1.0, bias=bias_t[:BL, 1:2])
            nc.sync.dma_start(od[:], osb[:])
    nc.compile()
    return nc


def _host_inputs(inputs):
    x = np.ascontiguousarray(inputs["x"], np.float32)
    cond = np.ascontiguousarray(inputs["cond"], np.float32)
    bo2 = np.asarray(inputs["bo2"], np.float32)

    bands, off, NP = _build_bands(
        np.asarray(inputs["Wci"], np.float32),
        np.asarray(inputs["Wc1"], np.float32),
        np.asarray(inputs["Wc2"], np.float32),
        np.asarray(inputs["Wo1"], np.float32),
        np.asarray(inputs["Wo2"], np.float32),
        np.asarray(inputs["Wih"], np.float32),
        np.asarray(inputs["Whh"], np.float32),
        np.asarray(inputs["bci"], np.float32),
        np.asarray(inputs["bc1"], np.float32),
        np.asarray(inputs["bc2"], np.float32),
        np.asarray(inputs["bo1"], np.float32),
        bo2,
        np.asarray(inputs["bih"], np.float32))

    x8 = np.zeros((NCORES, C - 1, 2, H, FREE), F8)
    xb = np.zeros((NCORES, C, H, FREE), BF16)
    c8 = np.zeros((NCORES, 8, 2, H, FREE), F8)
    for core in range(NCORES):
        xs = x[core * BL:(core + 1) * BL]        # [8, C, H, W]
        cs = cond[core * BL:(core + 1) * BL]
        A = np.zeros((C, H, FREE), np.float32)
        Ac = np.zeros((C, H, FREE), np.float32)
        for b in range(BL):
            s = OFF + b * WB + 1
            A[:, :, s:s + 128] = xs[b]
            Ac[:, :, s:s + 128] = cs[b]
        A8 = A.astype(F8)
        x8[core, :, 0] = A8[:C - 1]
        x8[core, :C - 1, 1, :, :-1] = A8[:C - 1, :, 1:]
        xb[core] = (A - float(bo2[0])).astype(BF16)
        Ac8 = Ac.astype(F8)
        c8[core, :, 0] = Ac8[0::2]
        c8[core, :, 1] = Ac8[1::2]
    return x8, xb, c8, bands


def kernel(**inputs):
    x8, xb, c8, bands = _host_inputs(inputs)
    nc = _build_program(
        np.asarray(inputs["bci"], np.float32),
        np.asarray(inputs["bc1"], np.float32),
        np.asarray(inputs["bc2"], np.float32),
        np.asarray(inputs["bo1"], np.float32),
        np.asarray(inputs["bo2"], np.float32),
        np.asarray(inputs["bih"], np.float32))
    from concourse.bass_utils import run_bass_kernel_spmd
    in_maps = [
        {"x8": x8[i], "xb": xb[i], "c8": c8[i], "bands": bands}
        for i in range(NCORES)
    ]
    res = run_bass_kernel_spmd(nc, in_maps, list(range(NCORES)))
    out = np.concatenate(
        [res.results[i]["out"].reshape(BL) for i in range(NCORES)])
    return out.astype(np.float32)


if __name__ == "__main__":
    rng = np.random.default_rng(0)
    ins = {
        "x": rng.standard_normal((64, 16, 128, 128)).astype(np.float32),
        "cond": rng.standard_normal((64, 16, 128, 128)).astype(np.float32),
        "Wci": (rng.standard_normal((3, 3, 1, 1)) * 0.1).astype(np.float32),
        "bci": np.zeros(1, np.float32),
        "Wc1": (rng.standard_normal((3, 3, 16, 2)) * 0.1).astype(np.float32),
        "bc1": np.zeros(2, np.float32),
        "Wc2": (rng.standard_normal((3, 3, 2, 2)) * 0.1).astype(np.float32),
        "bc2": np.zeros(2, np.float32),
        "Wo1": (rng.standard_normal((3, 3, 4, 2)) * 0.1).astype(np.float32),
        "bo1": np.zeros(2, np.float32),
        "Wo2": (rng.standard_normal((3, 3, 2, 2)) * 0.1).astype(np.float32),
        "bo2": np.zeros(2, np.float32),
        "Wih": (rng.standard_normal((3, 3, 1, 8)) * 0.1).astype(np.float32),
        "bih": np.zeros(8, np.float32),
        "Whh": (rng.standard_normal((3, 3, 2, 8)) * 0.1).astype(np.float32),
    }
    print(kernel(**ins)[:8])


# revision 60
# speedup vs baseline: 1.0024x; 1.0024x over previous
"""Trainium2 Bass kernel for AutoregressiveConvLSTM log-prob.

Strategy (v2)
-------------
Data-parallel over batch: 64 images -> 8 NeuronCores, 8 images each.

Layout: each plane is [H=128 partitions, FREE] where image b occupies
flat columns OFF+130*b .. OFF+130*b+129 (interior at +1..+128, one zero
pad column each side; OFF=2 leading zeros allow dx=-2 taps).

All 3x3 convs run on the TensorEngine as banded matmuls in fp8(e4m3)
with MatmulPerfMode.DoubleRow: each instruction computes
  psum += bandA.T @ movingA + bandB.T @ movingB
at 0.5 PE cycles per output column (4x the fp32r rate).  Band pairs are
host-built [128, 2, 128] fp8 tri/penta-diagonal matrices.  The dy taps
live in the band diagonals; dx taps are free-dim column offsets into
the zero pads.  Pair sources must share one SBUF tile, so the state
pack P = [128, 6, FREE] holds (h0, h1, cf0, cf1, r0, r1) and the x
stream holds (x, x-shifted-left-1) so taps pair across dx.  The
conv_in (1->1) conv is folded into Wih as a single 5x5 conv (exact for
bci=0; interior-exact otherwise), removing the u plane entirely.

Sigmoids are computed as 0.5*tanh(x/2)+0.5 (Act tanh + DVE
tensor_scalar) so every activation comes from one table set - no
LoadActFuncSet thrash.  Gate psums are [128, 2, 512] (co-pairs fused)
so one Act op covers both features.  LSTM pointwise math runs in bf16
on DVE (2x mode); h-writes (bf16*bf16->fp8) run on the idle Pool
engine.  Per-pixel log-prob terms reduce via tensor_tensor_reduce with
the lp column as both init and accumulator.
"""

import numpy as np
import ml_dtypes

B_FULL, C, H, W, F = 64, 16, 128, 128, 2
NCORES = 8
BL = B_FULL // NCORES            # images per core
WB = W + 2                       # per-image block width incl pads
OFF = 2                          # leading zero cols (dx=-2 reach)
FREE = OFF + BL * WB + 2
HALF_LOG_2PI = 0.9189385332046727
LN_SQRT2 = 0.34657359027997264

F8 = ml_dtypes.float8_e4m3
BF16 = ml_dtypes.bfloat16

# chunks: (b0, n_imgs); psum free cols = n*130
CHUNKS = [(0, 3), (3, 3), (6, 2)]


def _nz(v):
    return float(v) != 0.0


def _pair_layout(bci, bc1, bc2, bo1, bo2, bih):
    """Ordered (key -> (offset, count)) for the band-pair DRAM tensor.
    Depends only on which biases are nonzero, so the program builder can
    mirror it without the weights."""
    gb = [_nz(bih[g]) or _nz(bci[0]) for g in range(8)]
    L = []
    for co in range(2):
        L.append((f"c1_{co}", 24 + (1 if _nz(bc1[co]) else 0)))
    for co in range(2):
        L.append((f"c2_{co}", 3 + (1 if _nz(bc2[co]) else 0)))
    for g in range(8):
        L.append((f"g{g}", 6 + (1 if gb[g] else 0)))
    for co in range(2):
        L.append((f"h1_{co}", 6 + (1 if _nz(bo1[co]) else 0)))
    for co in range(2):
        L.append((f"h2_{co}", 3))
    off = {}
    o = 0
    for k, n in L:
        off[k] = (o, n)
        o += n
    return off, o


def _band3(w3):
    b = np.zeros((H, H), np.float32)
    for dy in (-1, 0, 1):
        ar = np.arange(max(0, -dy), H - max(0, dy))
        b[ar + dy, ar] = w3[dy + 1]
    return b


def _band5(w5):
    b = np.zeros((H, H), np.float32)
    for dy in (-2, -1, 0, 1, 2):
        ar = np.arange(max(0, -dy), H - max(0, dy))
        b[ar + dy, ar] = w5[dy + 2]
    return b


def _bias_band(v):
    b = np.zeros((H, H), np.float32)
    b[0, :] = v
    return b


_ZB = np.zeros((H, H), np.float32)


def _build_bands(Wci, Wc1, Wc2, Wo1, Wo2, Wih, Whh,
                 bci, bc1, bc2, bo1, bo2, bih):
    off, total = _pair_layout(bci, bc1, bc2, bo1, bo2, bih)
    bands = np.zeros((total, H, 2, H), np.float32)
    pos = {k: o for k, (o, n) in off.items()}

    def emit(key, a, b):
        i = pos[key]
        bands[i, :, 0, :] = a
        bands[i, :, 1, :] = b
        pos[key] = i + 1

    # cond1: 16 -> 2; channel pairs (2k, 2k+1)
    for co in range(2):
        k0 = f"c1_{co}"
        for k in range(8):
            for dx in range(3):
                emit(k0, _band3(Wc1[:, dx, 2 * k, co]),
                     _band3(Wc1[:, dx, 2 * k + 1, co]))
        if _nz(bc1[co]):
            emit(k0, _bias_band(bc1[co]), _ZB)
    # cond2: 2 -> 2
    for co in range(2):
        k0 = f"c2_{co}"
        for dx in range(3):
            emit(k0, _band3(Wc2[:, dx, 0, co]), _band3(Wc2[:, dx, 1, co]))
        if _nz(bc2[co]):
            emit(k0, _bias_band(bc2[co]), _ZB)
    # gates: 5x5 composite of Wci then Wih, plus Whh
    W5 = np.zeros((5, 5, 8), np.float32)
    for co in range(8):
        for a in range(3):
            for d in range(3):
                for b in range(3):
                    for e in range(3):
                        W5[a + b, d + e, co] += (
                            Wci[a, d, 0, 0] * Wih[b, e, 0, co])
    gbias = [float(bih[co]) + float(bci[0]) * float(Wih[:, :, 0, co].sum())
             for co in range(8)]
    for co in range(8):
        k0 = f"g{co}"
        for dx in range(3):
            emit(k0, _band3(Whh[:, dx, 0, co]), _band3(Whh[:, dx, 1, co]))
        emit(k0, _band5(W5[:, 0, co]), _band5(W5[:, 1, co]))   # xbase -2
        emit(k0, _band5(W5[:, 2, co]), _band5(W5[:, 3, co]))   # xbase 0
        emit(k0, _band5(W5[:, 4, co]), _ZB)                     # xbase +2
        if _nz(bih[co]) or _nz(bci[0]):
            emit(k0, _bias_band(gbias[co]), _ZB)
    # head1: feat part + cond part of Wo1
    for co in range(2):
        k0 = f"h1_{co}"
        for dx in range(3):
            emit(k0, _band3(Wo1[:, dx, 0, co]), _band3(Wo1[:, dx, 1, co]))
        for dx in range(3):
            emit(k0, _band3(Wo1[:, dx, 2, co]), _band3(Wo1[:, dx, 3, co]))
        if _nz(bo1[co]):
            emit(k0, _bias_band(bo1[co]), _ZB)
    # head2
    for co in range(2):
        k0 = f"h2_{co}"
        for dx in range(3):
            emit(k0, _band3(Wo2[:, dx, 0, co]), _band3(Wo2[:, dx, 1, co]))
    for k, (o, n) in off.items():
        assert pos[k] == o + n, (k, pos[k], o, n)
    return np.ascontiguousarray(
        bands.astype(F8).transpose(1, 0, 2, 3)), off, total


def _build_program(bci, bc1, bc2, bo1, bo2, bih):
    import concourse.bacc as bacc
    import concourse.mybir as mybir
    import concourse.tile as tile

    f32 = mybir.dt.float32
    f8 = mybir.dt.float8e4
    bf = mybir.dt.bfloat16
    AF = mybir.ActivationFunctionType
    OP = mybir.AluOpType
    AX = mybir.AxisListType
    DR = mybir.MatmulPerfMode.DoubleRow

    off, NP = _pair_layout(bci, bc1, bc2, bo1, bo2, bih)
    n_ot = off["g0"][0]                      # one-time pairs (cond)
    n_res = NP - n_ot                        # resident pairs

    nc = bacc.Bacc("TRN2", target_bir_lowering=False, debug=False)
    xd8 = nc.dram_tensor("x8", [C - 1, 2, H, FREE], f8, kind="ExternalInput")
    xbd = nc.dram_tensor("xb", [C, H, FREE], bf, kind="ExternalInput")
    cdd = nc.dram_tensor("c8", [8, 2, H, FREE], f8, kind="ExternalInput")
    bdd = nc.dram_tensor("bands", [H, NP, 2, H], f8, kind="ExternalInput")
    od = nc.dram_tensor("out", [BL, 1], f32, kind="ExternalOutput")

    def BS(b):
        return OFF + b * WB

    with tile.TileContext(nc) as tc:
        import contextlib
        ctx = contextlib.ExitStack()
        with ctx:
            state = ctx.enter_context(tc.tile_pool(name="state", bufs=1))
            sbands = ctx.enter_context(tc.tile_pool(name="sbands", bufs=1))
            xstream = ctx.enter_context(tc.tile_pool(name="xs", bufs=4))
            bstream = ctx.enter_context(tc.tile_pool(name="bs", bufs=4))
            tmp = ctx.enter_context(tc.tile_pool(name="tmp", bufs=20))
            psum = ctx.enter_context(
                tc.tile_pool(name="psum", bufs=4, space="PSUM"))
            psumH = psum

            # resident band pairs (loaded after the cond-phase DMAs below)
            sb = sbands.tile([H, n_res, 2, H], f8, tag="sb", name="sb")

            def bp(key, j):
                o, n = off[key]
                assert j < n
                return sb[:, o - n_ot + j]

            # persistent state; slots: 0,1 h(even-step) / 2,3 cf /
            # 4,5 r / 6,7 h(odd-step)  (h ping-pongs so delayed heads can
            # read the previous step's h)
            P = state.tile([H, 8, FREE], f8, tag="P", name="P")
            nc.gpsimd.memset(P[:], 0.0)
            cst_t = state.tile([H, 2, FREE], bf, tag="c", name="c")
            nc.vector.memset(cst_t[:], 0.0)
            ones8 = state.tile([H, 2, 258], f8, tag="o8", name="o8")
            nc.vector.memset(ones8[:], 1.0)
            lp = state.tile([H, BL], f32, tag="lp", name="lp")
            nc.vector.memset(lp[:], 0.0)
            ones_f = state.tile([H, 1], f32, tag="of", name="of")
            nc.vector.memset(ones_f[:], 1.0)
            # bias cols: 0 = exp bias, 1 = final output bias
            cstv = -16.0 * 128.0 * 128.0 * (float(bo2[1]) + HALF_LOG_2PI)
            bias_t = state.tile([H, 2], f32, tag="bias", name="bias")
            nc.vector.memset(bias_t[:, 0:1], -float(bo2[1]) - LN_SQRT2)
            nc.vector.memset(bias_t[:, 1:2], cstv)

            def interior(ap_flat):
                # [p, s, NN] -> [p, s, n, 128]
                return ap_flat.rearrange("p s (b w) -> p s b w", w=WB)[
                    :, :, :, 1:129]

            def regions(NN):
                # split a chunk's columns into <=256-wide matmul regions
                # (regions need not align to images; pads absorb conv
                # bleed).  Avoid degenerate tiny regions: split evenly when
                # the remainder would be very small.
                if NN <= 256:
                    return [(0, NN)]
                if NN - 256 >= 64:
                    return [(0, 256), (256, NN - 256)]
                h = NN // 2
                return [(0, h), (h, NN - h)]

            def head(key_pfx, src_slots, Tg, b0, n, include_h=True,
                     bias_flags=(False, False)):
                # head1/head2-style group: co at dim1 of Tg.  For head1 the
                # h-independent cond-pairs (idx 3..5) are emitted FIRST so
                # they can fill PE bubbles while the h chain completes.
                NN = n * 130
                for co in range(2):
                    key = f"{key_pfx}_{co}"
                    for r0, rl in regions(NN):
                        base = BS(b0) + r0
                        out = Tg[:, co, r0:r0 + rl]
                        order = []
                        if key_pfx == "h1":
                            for dxi, dx in enumerate((-1, 0, 1)):
                                order.append((3 + dxi, 2, dx))
                            if include_h:
                                for dxi, dx in enumerate((-1, 0, 1)):
                                    order.append((dxi, src_slots[0], dx))
                        else:
                            for dxi, dx in enumerate((-1, 0, 1)):
                                order.append((dxi, src_slots[0], dx))
                        if bias_flags[co]:
                            order.append((off[key][1] - 1, None, None))
                        for k, (idx, slot, dx) in enumerate(order):
                            first = (k == 0)
                            last = (k == len(order) - 1)
                            if slot is None:
                                nc.tensor.matmul(
                                    out, bp(key, idx), ones8[:, :, 0:rl],
                                    start=first, stop=last, perf_mode=DR)
                            else:
                                nc.tensor.matmul(
                                    out, bp(key, idx),
                                    P[:, slot:slot + 2,
                                      base + dx:base + dx + rl],
                                    start=first, stop=last, perf_mode=DR)

            def head2_and_lp(b0, n, xbt, fast_tail=False):
                NN = n * 130
                eng = nc.vector if fast_tail else nc.gpsimd
                Th2 = psumH.tile([H, 2, 512], f32, tag="ps", name="ps")
                head("h2", (4, 4), Th2, b0, n)
                e = tmp.tile([H, NN], bf, tag="tw", name="e")
                nc.scalar.activation(e[:], Th2[:, 1, :NN], AF.Exp,
                                     bias=bias_t[:, 0:1], scale=-1.0)
                d2 = tmp.tile([H, NN], bf, tag="tw", name="d2")
                nc.vector.tensor_tensor(
                    d2[:], xbt[:, BS(b0):BS(b0) + NN], Th2[:, 0, :NN],
                    OP.subtract)
                # copy pq1 to SBUF so the head2 psum tile releases early
                ls = tmp.tile([H, NN], bf, tag="tw", name="ls")
                nc.vector.tensor_scalar(ls[:], Th2[:, 1, :NN], 0.0, None,
                                        OP.add)
                z = tmp.tile([H, NN], bf, tag="tw", name="z")
                eng.tensor_tensor(z[:], d2[:], e[:], OP.mult)
                z2 = tmp.tile([H, NN], bf, tag="tw", name="z2")
                eng.tensor_tensor(z2[:], z[:], z[:], OP.mult)
                t = tmp.tile([H, NN], bf, tag="tw", name="t")
                nc.vector.tensor_tensor(t[:], z2[:], ls[:], OP.add)
                red = tmp.tile([H, n], f32, tag="tw", name="red")
                t3 = t[:].rearrange("p (b w) -> p b w", w=WB)[:, :, 1:129]
                nc.vector.reduce_sum(red[:], t3, AX.X)
                nc.vector.tensor_add(lp[:, b0:b0 + n], lp[:, b0:b0 + n],
                                     red[:])

            # ---------------- cond phase ----------------
            with tc.tile_pool(name="otb", bufs=1) as otp, \
                 tc.tile_pool(name="cstr", bufs=1) as cstr:
                ot = otp.tile([H, n_ot, 2, H], f8, tag="ot", name="ot")

                def bot(key, j):
                    o, n = off[key]
                    assert j < n
                    return ot[:, o + j]

                tc8 = state.tile([H, 2, FREE], f8, tag="tc8", name="tc8")
                nc.vector.memset(tc8[:], 0.0)

                # all 8 cond channel-pair planes resident
                cpl = cstr.tile([H, 8, 2, FREE], f8, tag="cpl", name="cpl")
                n_c1 = off["c2_0"][0]          # cond1 pairs come first
                nc.sync.dma_start(ot[:, :n_c1], bdd[:, :n_c1])
                for k in range(8):
                    nc.sync.dma_start(
                        cpl[:, k], cdd[k].rearrange("t h w -> h t w"))
                # prefetch step-1 inputs and the gate bands so step 1's gate
                # phase can overlap the cond phase tail
                xpl1 = xstream.tile([H, 2, FREE], f8, tag="xpl", name="xpl")
                nc.sync.dma_start(xpl1[:], xd8[0].rearrange("t h w -> h t w"))
                sb_g = off["h1_0"][0] - n_ot   # gate pairs precede head pairs
                nc.sync.dma_start(sb[:, :sb_g], bdd[:, n_ot:n_ot + sb_g])
                nc.sync.dma_start(ot[:, n_c1:], bdd[:, n_c1:n_ot])
                nc.sync.dma_start(sb[:, sb_g:], bdd[:, n_ot + sb_g:])
                for ci, (b0, n) in enumerate(CHUNKS):
                    NN = n * 130
                    pc = psum.tile([H, 2, 512], f32, tag="ps", name="ps")
                    for co in range(2):
                        key = f"c1_{co}"
                        tot = off[key][1]
                        for r0, rl in regions(NN):
                            base = BS(b0) + r0
                            out = pc[:, co, r0:r0 + rl]
                            for k in range(8):
                                for dxi, dx in enumerate((-1, 0, 1)):
                                    idx = k * 3 + dxi
                                    nc.tensor.matmul(
                                        out, bot(key, idx),
                                        cpl[:, k, :,
                                            base + dx:base + dx + rl],
                                        start=(idx == 0),
                                        stop=(idx == tot - 1),
                                        perf_mode=DR)
                            if _nz(bc1[co]):
                                nc.tensor.matmul(
                                    out, bot(key, tot - 1),
                                    ones8[:, :, 0:rl],
                                    start=False, stop=True, perf_mode=DR)
                    nc.scalar.activation(
                        interior(tc8[:, :, BS(b0):BS(b0) + NN]),
                        interior(pc[:, :, :NN]), AF.Tanh)
                # cond2 -> cf slots of P
                for ci, (b0, n) in enumerate(CHUNKS):
                    NN = n * 130
                    pq = psum.tile([H, 2, 512], f32, tag="ps", name="ps")
                    for co in range(2):
                        key = f"c2_{co}"
                        tot = off[key][1]
                        for r0, rl in regions(NN):
                            base = BS(b0) + r0
                            out = pq[:, co, r0:r0 + rl]
                            for dxi, dx in enumerate((-1, 0, 1)):
                                nc.tensor.matmul(
                                    out, bot(key, dxi),
                                    tc8[:, :, base + dx:base + dx + rl],
                                    start=(dxi == 0), stop=(dxi == tot - 1),
                                    perf_mode=DR)
                            if _nz(bc2[co]):
                                nc.tensor.matmul(
                                    out, bot(key, tot - 1),
                                    ones8[:, :, 0:rl],
                                    start=False, stop=True, perf_mode=DR)
                    nc.scalar.activation(
                        interior(P[:, 2:4, BS(b0):BS(b0) + NN]),
                        interior(pq[:, :, :NN]), AF.Identity)

            # ---------------- step 0 ----------------
            xbt0 = bstream.tile([H, FREE], bf, tag="xbt", name="xbt")
            nc.sync.dma_start(xbt0[:], xbd[0])
            h1b = (_nz(bo1[0]), _nz(bo1[1]))
            for (b0, n) in CHUNKS:
                NN = n * 130
                Th1 = psumH.tile([H, 2, 512], f32, tag="ps", name="ps")
                head("h1", (0, 0), Th1, b0, n, include_h=False,
                     bias_flags=h1b)
                nc.vector.tensor_scalar(
                    interior(P[:, 4:6, BS(b0):BS(b0) + NN]),
                    interior(Th1[:, :, :NN]), 0.0, None, OP.max)
                head2_and_lp(b0, n, xbt0)

            # ---------------- steps (phase-major across chunks) ----------
            gb = [_nz(bih[g]) or _nz(bci[0]) for g in range(8)]
            pend_CD = None
            for st in range(1, 16):
                xpl = xstream.tile([H, 2, FREE], f8, tag="xpl", name="xpl")
                nc.sync.dma_start(xpl[:], xd8[st - 1].rearrange(
                    "t h w -> h t w"))
                xbt = bstream.tile([H, FREE], bf, tag="xbt", name="xbt")
                nc.sync.dma_start(xbt[:], xbd[st])
                # Software-pipelined emission: one-chunk lag between the
                # gate phase (A), pointwise chain (B), head1 (C), head2 (D)
                # so every engine's in-order queue stays fed.
                st_tnh = {}
                st_sig = {}

                hs_r = 0 if (st - 1) % 2 == 0 else 6
                hs_w = 0 if st % 2 == 0 else 6

                def phA(ci):
                    b0, n = CHUNKS[ci]
                    NN = n * 130
                    tnh = []
                    sig = {}
                    for g in range(4):
                        Tg = psum.tile([H, 2, 512], f32, tag="ps", name="ps")
                        for f01 in range(2):
                            co = 2 * g + f01
                            key = f"g{co}"
                            # x-pairs (h-independent) first: they can fill PE
                            # bubbles while the h recurrence chain completes
                            for r0, rl in regions(NN):
                                base = BS(b0) + r0
                                out = Tg[:, f01, r0:r0 + rl]
                                for xi, xb_ in enumerate((-2, 0, 2)):
                                    nc.tensor.matmul(
                                        out, bp(key, 3 + xi),
                                        xpl[:, :, base + xb_:base + xb_ + rl],
                                        start=(xi == 0), stop=False,
                                        perf_mode=DR)
                            for r0, rl in regions(NN):
                                base = BS(b0) + r0
                                out = Tg[:, f01, r0:r0 + rl]
                                for dxi, dx in enumerate((-1, 0, 1)):
                                    nc.tensor.matmul(
                                        out, bp(key, dxi),
                                        P[:, hs_r:hs_r + 2,
                                          base + dx:base + dx + rl],
                                        start=False,
                                        stop=(dxi == 2 and not gb[co]),
                                        perf_mode=DR)
                                if gb[co]:
                                    nc.tensor.matmul(
                                        out, bp(key, off[key][1] - 1),
                                        ones8[:, :, 0:rl],
                                        start=False, stop=True, perf_mode=DR)
                        t = tmp.tile([H, 2, NN], bf, tag="tw", name="tnh")
                        nc.scalar.activation(
                            t[:], Tg[:, :, :NN], AF.Tanh,
                            scale=(1.0 if g == 2 else 0.5))
                        tnh.append(t)
                        if g != 2:
                            s = tmp.tile([H, 2, NN], bf, tag="tw", name="sig")
                            nc.vector.tensor_scalar(
                                s[:], t[:], 0.5, 0.5, OP.mult, OP.add)
                            sig[g] = s
                    st_tnh[ci] = tnh
                    st_sig[ci] = sig

                def phB(ci):
                    b0, n = CHUNKS[ci]
                    NN = n * 130
                    tnh, sig = st_tnh[ci], st_sig[ci]
                    tig = tmp.tile([H, 2, NN], bf, tag="tw", name="tig")
                    nc.vector.tensor_tensor(
                        tig[:], sig[0][:], tnh[2][:], OP.mult)
                    cs = cst_t[:, :, BS(b0):BS(b0) + NN]
                    nc.vector.tensor_tensor(cs, sig[1][:], cs, OP.mult)
                    nc.vector.tensor_tensor(cs, cs, tig[:], OP.add)
                    tcc = tmp.tile([H, 2, NN], bf, tag="tw", name="tcc")
                    nc.scalar.activation(tcc[:], cs, AF.Tanh)
                    nc.vector.tensor_tensor(
                        interior(P[:, hs_w:hs_w + 2, BS(b0):BS(b0) + NN]),
                        interior(sig[3][:]), interior(tcc[:]), OP.mult)

                def phC(ci, hs_=hs_w):
                    b0, n = CHUNKS[ci]
                    NN = n * 130
                    Th1 = psumH.tile([H, 2, 512], f32, tag="ps", name="ps")
                    head("h1", (hs_, 0), Th1, b0, n, bias_flags=h1b)
                    nc.scalar.activation(
                        interior(P[:, 4:6, BS(b0):BS(b0) + NN]),
                        interior(Th1[:, :, :NN]), AF.Relu)

                def phD(ci, xbt_=None, fast_tail=False):
                    b0, n = CHUNKS[ci]
                    head2_and_lp(b0, n, xbt_ if xbt_ is not None else xbt,
                                 fast_tail=fast_tail)

                for ci in range(3):
                    phA(ci)
                for ci in range(3):
                    phB(ci)
                # heads of the PREVIOUS step (one-step lag): their PE work
                # fills this step's recurrence-chain bubble, and next-step
                # gate psum tiles can allocate before head tiles
                if pend_CD is not None:
                    pC, pD = pend_CD
                    for ci in range(3):
                        pC(ci)
                    for ci in range(3):
                        pD(ci)
                pend_CD = (phC,
                           (lambda ci, xb_=xbt, ft=False: phD(ci, xb_, ft)))
            # drain the last step's heads
            pC, pD = pend_CD
            for ci in range(3):
                pC(ci)
                pD(ci, None, True)

            # final (reuse a psum-pool tile for the [8,1] reduction)
            po_t = psumH.tile([H, 2, 512], f32, tag="ps", name="ps")
            po = po_t[:BL, 0, 0:1]
            nc.tensor.matmul(po, lp[:], ones_f[:], start=True, stop=True)
            osb = state.tile([BL, 1], f32, tag="osb", name="osb")
            nc.scalar.activation(osb[:], po, AF.Identity,
                                 scale=-1.0, bias=bias_t[:BL, 1:2])
            nc.sync.dma_start(od[:], osb[:])
    nc.compile()
    return nc


def _host_inputs(inputs):
    x = np.ascontiguousarray(inputs["x"], np.float32)
    cond = np.ascontiguousarray(inputs["cond"], np.float32)
    bo2 = np.asarray(inputs["bo2"], np.float32)

    bands, off, NP = _build_bands(
        np.asarray(inputs["Wci"], np.float32),
        np.asarray(inputs["Wc1"], np.float32),
        np.asarray(inputs["Wc2"], np.float32),
        np.asarray(inputs["Wo1"], np.float32),
        np.asarray(inputs["Wo2"], np.float32),
        np.asarray(inputs["Wih"], np.float32),
        np.asarray(inputs["Whh"], np.float32),
        np.asarray(inputs["bci"], np.float32),
        np.asarray(inputs["bc1"], np.float32),
        np.asarray(inputs["bc2"], np.float32),
        np.asarray(inputs["bo1"], np.float32),
        bo2,
        np.asarray(inputs["bih"], np.float32))

    x8 = np.zeros((NCORES, C - 1, 2, H, FREE), F8)
    xb = np.zeros((NCORES, C, H, FREE), BF16)
    c8 = np.zeros((NCORES, 8, 2, H, FREE), F8)
    for core in range(NCORES):
        xs = x[core * BL:(core + 1) * BL]        # [8, C, H, W]
        cs = cond[core * BL:(core + 1) * BL]
        A = np.zeros((C, H, FREE), np.float32)
        Ac = np.zeros((C, H, FREE), np.float32)
        for b in range(BL):
            s = OFF + b * WB + 1
            A[:, :, s:s + 128] = xs[b]
            Ac[:, :, s:s + 128] = cs[b]
        A8 = A.astype(F8)
        x8[core, :, 0] = A8[:C - 1]
        x8[core, :C - 1, 1, :, :-1] = A8[:C - 1, :, 1:]
        xb[core] = (A - float(bo2[0])).astype(BF16)
        Ac8 = Ac.astype(F8)
        c8[core, :, 0] = Ac8[0::2]
        c8[core, :, 1] = Ac8[1::2]
    return x8, xb, c8, bands


def kernel(**inputs):
    x8, xb, c8, bands = _host_inputs(inputs)
    nc = _build_program(
        np.asarray(inputs["bci"], np.float32),
        np.asarray(inputs["bc1"], np.float32),
        np.asarray(inputs["bc2"], np.float32),
        np.asarray(inputs["bo1"], np.float32),
        np.asarray(inputs["bo2"], np.float32),
        np.asarray(inputs["bih"], np.float32))
    from concourse.bass_utils import run_bass_kernel_spmd
    in_maps = [
        {"x8": x8[i], "xb": xb[i], "c8": c8[i], "bands": bands}
        for i in range(NCORES)
    ]
    res = run_bass_kernel_spmd(nc, in_maps, list(range(NCORES)))
    out = np.concatenate(
        [res.results[i]["out"].reshape(BL) for i in range(NCORES)])
    return out.astype(np.float32)


if __name__ == "__main__":
    rng = np.random.default_rng(0)
    ins = {
        "x": rng.standard_normal((64, 16, 128, 128)).astype(np.float32),
        "cond": rng.standard_normal((64, 16, 128, 128)).astype(np.float32),
        "Wci": (rng.standard_normal((3, 3, 1, 1)) * 0.1).astype(np.float32),
        "bci": np.zeros(1, np.float32),
        "Wc1": (rng.standard_normal((3, 3, 16, 2)) * 0.1).astype(np.float32),
        "bc1": np.zeros(2, np.float32),
        "Wc2": (rng.standard_normal((3, 3, 2, 2)) * 0.1).astype(np.float32),
        "bc2": np.zeros(2, np.float32),
        "Wo1": (rng.standard_normal((3, 3, 4, 2)) * 0.1).astype(np.float32),
        "bo1": np.zeros(2, np.float32),
        "Wo2": (rng.standard_normal((3, 3, 2, 2)) * 0.1).astype(np.float32),
        "bo2": np.zeros(2, np.float32),
        "Wih": (rng.standard_normal((3, 3, 1, 8)) * 0.1).astype(np.float32),
        "bih": np.zeros(8, np.float32),
        "Whh": (rng.standard_normal((3, 3, 2, 8)) * 0.1).astype(np.float32),
    }
    print(kernel(**ins)[:8])
